# revision 8
# baseline (speedup 1.0000x reference)
"""Trainium2 Bass kernel for nn_LoRALinear (out = x @ (W + s*L@R)^T + bias).

Full shapes: x [4, 2048, 4096], weight [4096, 4096], bias [4096],
lora_left [4096, 16], lora_right [16, 4096], out [4, 2048, 4096].

Sharding (8 cores, 2D): tokens split 4 ways (the batch dim) x d_out split
2 ways. Core i handles batch b = i % 4 and output half oh = i // 4, i.e. a
[2048, 2048] output block with the full K = 4096 contraction.

Host-side layout prep (part of sharding): the TensorEngine contracts over
the partition dim of both operands, so both x and W need d_in-major
layouts; fp32 has no DMA-transpose path on trn2. We pre-transpose the
shards on the host (np.ascontiguousarray) so the device kernel is pure
matmul. The LoRA right factor is pre-transposed too, and the scaled L^T is
shipped scaled; bias is shipped replicated to 128 partitions so the
kh=0 PSUM->SBUF copy doubles as the bias add, and the LoRA term joins the
kh=1 PSUM accumulation group as one extra 16-partition matmul.

Device blocking (per core, ~176KB/partition usable SBUF):
  loops kh in {0,1} (K halves of 2048) x tb in {0,1} (token halves of 1024);
  x^T[kh, tb] resident in SBUF ([128, 16, 1024] = 64KB/part, read once),
  w^T[kh] streamed in [128, 16, 512] o-chunks (32KB/part, double-buffered,
  read once per tb so twice overall);
  psum [128, 512] accumulates 16 matmuls; kh=0 writes partials to a DRAM
  scratch tile, kh=1 adds them back in.
"""

import os
import sys

import numpy as np

for _p in ("/root/.axon_site/_ro/trn_rl_repo", "/opt/trn_rl_repo"):
    if _p not in sys.path and os.path.isdir(_p):
        sys.path.append(_p)

import bass_rust
import concourse.bass as bass
import concourse.mybir as mybir
import concourse.tile as tile
from concourse.bass import ts
from concourse.bass_utils import run_bass_kernel_spmd
from concourse.vector_clock import ScopedClock, VectorClock

# ---- problem constants (hardcoded per contract) ----
B, S, D_IN, D_OUT, LORA_DIM = 4, 2048, 4096, 4096, 16
LORA_SCALE = 32.0 / LORA_DIM
N_CORES = 8
T = 2048          # tokens per core (= one batch element)
O = 2048          # d_out per core (half)
K = D_IN          # contraction
KH = K // 2       # K half resident in SBUF
NKT = KH // 128   # 16 k-tiles per half
TB = 1024         # token block
NTB = T // TB     # 2 token blocks
NTT_B = TB // 128  # 8 token tiles per block
OCW = 512         # o-chunk width (one PSUM bank)
NOC = O // OCW    # 4 o-chunks

# "f32r" (full-rate fp32, reduced-precision multiplies) or "f32" (exact fp32,
# 4 cycles/row). Flip via env for A/B testing; default chosen empirically.
COMPUTE = os.environ.get("LORA_KERNEL_COMPUTE", "f32r")

# Set by kernel() after a traced run (test.py reads it).
LAST_EXEC_TIME_NS = None
TRACE = False


class SplitDrainTileContext(tile.TileContext):
    """TileContext that splits multi-wait instructions for this walrus build.

    This walrus rejects instructions carrying >2 sync waits ("Too many sync
    wait commands"). Engine queues are in-order, so an instruction's waits
    can equivalently ride same-engine NOPs inserted just before it; we cap
    every instruction at one wait. Same treatment for the exit Drain.
    """

    _splitw_counter = 0

    def _split_excess_waits(self, ordered):
        for bb_name, insts in ordered.items():
            new_list = []
            changed = False
            for inst in insts:
                si = getattr(inst, "sync_info", None)
                eng = getattr(inst, "engine", mybir.EngineType.Unassigned)
                waits = list(si.on_wait) if si is not None and si.on_wait else []
                if len(waits) > 1 and eng != mybir.EngineType.Unassigned:
                    # keep register-valued waits (if any) on the original
                    movable = [w for w in waits if w.wait_reg is None]
                    pinned = [w for w in waits if w.wait_reg is not None]
                    keep = pinned + movable[-1:] if not pinned else pinned
                    move = movable[:-1] if not pinned else movable
                    for w in move:
                        SplitDrainTileContext._splitw_counter += 1
                        nop = bass_rust.InstNoOp(
                            name=f"tile_splitw_{SplitDrainTileContext._splitw_counter}",
                            ins=[],
                            outs=[],
                        )
                        nop.engine = eng
                        nop.sync_info = bass_rust.SyncInfo(
                            on_wait=[w], on_update=[]
                        )
                        new_list.append(nop)
                    inst.sync_info = bass_rust.SyncInfo(
                        on_wait=keep, on_update=list(si.on_update)
                    )
                    changed = True
                new_list.append(inst)
            if changed:
                insts[:] = new_list

    def _lower_ordered_insts(self, ordered):
        self._split_excess_waits(ordered)
        return super()._lower_ordered_insts(ordered)

    def _drain_and_barrier(self, tick_clock, wait_clock):
        g = tick_clock.global_clock
        for proc in range(len(g)):
            t = g[proc]
            if t <= 0:
                continue
            v = VectorClock()
            v.require_at_least(proc, t)
            nop = self.nc.sync.nop(nofuse=True)
            wait_clock.add_sem_waits(nop.ins, ScopedClock({None: v}))
        drain_inst = self.nc.sync.drain()
        wait_clock.add_sem_waits(
            drain_inst.ins, ScopedClock({None: g}), ScopedClock({None: g})
        )
        self.nc.all_engine_barrier()
        assert self.sems is not None
        popped = self.nc._tile_sem_poison_stack.pop()
        assert popped is self._sem_poison
        self.nc.clear_and_free_semaphores(list(self.sems.allocated().values()))
        self.nc.all_engine_barrier()


def _build_nc() -> bass.Bass:
    f32 = mybir.dt.float32
    mm_dt = mybir.dt.float32r if COMPUTE == "f32r" else f32

    nc = bass.Bass("TRN2", target_bir_lowering=False, debug=False)
    xT = nc.declare_dram_parameter("xT", [K, T], f32, isOutput=False)
    wT = nc.declare_dram_parameter("wT", [K, O], f32, isOutput=False)
    rT = nc.declare_dram_parameter("rT", [K, LORA_DIM], f32, isOutput=False)
    lT = nc.declare_dram_parameter("lT", [LORA_DIM, O], f32, isOutput=False)
    biasr = nc.declare_dram_parameter("biasr", [128, O], f32, isOutput=False)
    out = nc.declare_dram_parameter("out", [T, O], f32, isOutput=True)

    def mm(ap):
        return ap.bitcast(mm_dt) if mm_dt is not f32 else ap

    with SplitDrainTileContext(nc) as tc:
        with (
            tc.tile_pool(name="xt", bufs=1) as xt_pool,
            tc.tile_pool(name="wt", bufs=2) as wt_pool,
            tc.tile_pool(name="consts", bufs=1) as const_pool,
            tc.tile_pool(name="outsb", bufs=2) as out_pool,
            tc.tile_pool(name="partsb", bufs=2) as part_pool,
            tc.tile_pool(name="psum", bufs=4, space="PSUM") as psum_pool,
            tc.tile_pool(name="psum1", bufs=2, space="PSUM") as psum1_pool,
            tc.tile_pool(name="dram", bufs=1, space="DRAM") as dram_pool,
        ):
            # constants: R^T (full K), [s*L^T; bias], xR^T-plus-ones row
            rt_sb = const_pool.tile([128, K // 128, LORA_DIM], f32)
            nc.sync.dma_start(
                rt_sb[:], rT.rearrange("(ko p) j -> p ko j", p=128)
            )
            lt_sb = const_pool.tile([LORA_DIM, O], f32)
            nc.sync.dma_start(lt_sb[:], lT[:])
            bias_sb = const_pool.tile([128, O], f32)
            nc.sync.dma_start(bias_sb[:], biasr[:])
            xr = const_pool.tile([LORA_DIM, T], f32)

            partial = dram_pool.tile([T, O], f32)

            for kh in range(2):
                for tb in range(NTB):
                    xt = xt_pool.tile([128, NKT, TB], f32, tag="xt")
                    x_src = xT[
                        kh * KH : (kh + 1) * KH, ts(tb, TB)
                    ].rearrange("(ko p) t -> p ko t", p=128)
                    for q in range(4):
                        nq = NKT // 4
                        nc.sync.dma_start(
                            xt[:, q * nq : (q + 1) * nq, :],
                            x_src[:, q * nq : (q + 1) * nq, :],
                        )

                    # stage A: xR^T[j, t] accumulated over kh into xr1 rows 0..15
                    for c in range(TB // 512):
                        tg = tb * (TB // 512) + c
                        p1 = psum1_pool.tile([LORA_DIM, 512], f32, tag="p1")
                        for k in range(NKT):
                            nc.tensor.matmul(
                                p1[:],
                                mm(rt_sb[:, kh * NKT + k, :]),
                                mm(xt[:, k, ts(c, 512)]),
                                start=(k == 0),
                                stop=(k == NKT - 1),
                            )
                        if kh == 0:
                            nc.vector.tensor_copy(xr[:, ts(tg, 512)], p1[:])
                        else:
                            nc.vector.tensor_add(
                                xr[:, ts(tg, 512)], xr[:, ts(tg, 512)], p1[:]
                            )

                    # main: psum[t-tile, o-chunk] over this K half
                    for oc in range(NOC):
                        wt = wt_pool.tile([128, NKT, OCW], f32, tag="wt")
                        nc.sync.dma_start(
                            wt[:],
                            wT[kh * KH : (kh + 1) * KH, ts(oc, OCW)].rearrange(
                                "(ko p) o -> p ko o", p=128
                            ),
                        )
                        for tt in range(NTT_B):
                            gt = tb * NTT_B + tt  # global token tile
                            ps = psum_pool.tile([128, OCW], f32, tag="ps")
                            for k in range(NKT):
                                nc.tensor.matmul(
                                    ps[:],
                                    mm(xt[:, k, ts(tt, 128)]),
                                    mm(wt[:, k, :]),
                                    start=(k == 0),
                                    stop=(kh == 0 and k == NKT - 1),
                                )
                            if kh == 0:
                                # bias-add rides the psum->SBUF copy
                                ob = out_pool.tile([128, OCW], f32, tag="ob")
                                nc.vector.tensor_add(
                                    ob[:], ps[:], bias_sb[:, ts(oc, OCW)]
                                )
                                nc.sync.dma_start(
                                    partial[ts(gt, 128), ts(oc, OCW)], ob[:]
                                )
                            else:
                                # LoRA + bias ride the same accumulation group
                                # (kept exact fp32: tiny op count).
                                nc.tensor.matmul(
                                    ps[:],
                                    xr[:, ts(gt, 128)],
                                    lt_sb[:, ts(oc, OCW)],
                                    start=False,
                                    stop=True,
                                )
                                pb = part_pool.tile([128, OCW], f32, tag="pb")
                                nc.sync.dma_start(
                                    pb[:], partial[ts(gt, 128), ts(oc, OCW)]
                                )
                                ob = out_pool.tile([128, OCW], f32, tag="ob")
                                nc.vector.tensor_add(ob[:], ps[:], pb[:])
                                nc.sync.dma_start(
                                    out[ts(gt, 128), ts(oc, OCW)], ob[:]
                                )
    return nc


def kernel(**inputs: np.ndarray) -> np.ndarray:
    global LAST_EXEC_TIME_NS

    x = np.ascontiguousarray(np.asarray(inputs["x"], dtype=np.float32))
    weight = np.asarray(inputs["weight"], dtype=np.float32)
    bias = np.asarray(inputs["bias"], dtype=np.float32)
    lora_left = np.asarray(inputs["lora_left"], dtype=np.float32)
    lora_right = np.asarray(inputs["lora_right"], dtype=np.float32)

    # host-side shard + layout prep
    xT_shards = [np.ascontiguousarray(x[b].T) for b in range(B)]
    wT_halves = [
        np.ascontiguousarray(weight[oh * O : (oh + 1) * O, :].T) for oh in range(2)
    ]
    rT = np.ascontiguousarray(lora_right.T)
    lT_halves = [
        np.ascontiguousarray(LORA_SCALE * lora_left[oh * O : (oh + 1) * O, :].T)
        for oh in range(2)
    ]
    bias_halves = [
        np.ascontiguousarray(
            np.broadcast_to(bias[None, oh * O : (oh + 1) * O], (128, O))
        )
        for oh in range(2)
    ]

    in_maps = []
    for i in range(N_CORES):
        b, oh = i % B, i // B
        in_maps.append(
            {
                "xT": xT_shards[b],
                "wT": wT_halves[oh],
                "rT": rT,
                "lT": lT_halves[oh],
                "biasr": bias_halves[oh],
            }
        )

    nc = _build_nc()
    res = run_bass_kernel_spmd(
        nc, in_maps, core_ids=list(range(N_CORES)), trace=TRACE
    )
    LAST_EXEC_TIME_NS = res.exec_time_ns

    out = np.empty((B, S, D_OUT), dtype=np.float32)
    for i in range(N_CORES):
        b, oh = i % B, i // B
        out[b, :, oh * O : (oh + 1) * O] = res.results[i]["out"]
    return out


# revision 9
# speedup vs baseline: 3.0358x; 3.0358x over previous
"""Trainium2 Bass kernel for nn_LoRALinear (out = x @ (W + s*L@R)^T + bias).

Full shapes: x [4, 2048, 4096], weight [4096, 4096], bias [4096],
lora_left [4096, 16], lora_right [16, 4096], out [4, 2048, 4096].

Sharding (8 cores, 2D): tokens split 4 ways (the batch dim) x d_out split
2 ways. Core i handles batch b = i % 4 and output half oh = i // 4, i.e. a
[2048, 2048] output block with the full K = 4096 contraction.

Host-side layout prep (part of sharding): the TensorEngine contracts over
the partition dim of both operands, so both x and W need d_in-major
layouts; fp32 has no DMA-transpose path on trn2. We pre-transpose the
shards on the host (np.ascontiguousarray) so the device kernel is pure
matmul. The LoRA right factor is pre-transposed too, and the scaled L^T is
shipped scaled; bias is shipped replicated to 128 partitions so the
kh=0 PSUM->SBUF copy doubles as the bias add, and the LoRA term joins the
kh=1 PSUM accumulation group as one extra 16-partition matmul.

Device blocking (per core, ~176KB/partition usable SBUF):
  loops kh in {0,1} (K halves of 2048) x tb in {0,1} (token halves of 1024);
  x^T[kh, tb] resident in SBUF ([128, 16, 1024] = 64KB/part, read once),
  w^T[kh] streamed in [128, 16, 512] o-chunks (32KB/part, double-buffered,
  read once per tb so twice overall);
  psum [128, 512] accumulates 16 matmuls; kh=0 writes partials to a DRAM
  scratch tile, kh=1 adds them back in.
"""

import os
import sys

import numpy as np

for _p in ("/root/.axon_site/_ro/trn_rl_repo", "/opt/trn_rl_repo"):
    if _p not in sys.path and os.path.isdir(_p):
        sys.path.append(_p)

import bass_rust
import concourse.bass as bass
import concourse.mybir as mybir
import concourse.tile as tile
from concourse.bass import ts
from concourse.bass_utils import run_bass_kernel_spmd
from concourse.vector_clock import ScopedClock, VectorClock

# ---- problem constants (hardcoded per contract) ----
B, S, D_IN, D_OUT, LORA_DIM = 4, 2048, 4096, 4096, 16
LORA_SCALE = 32.0 / LORA_DIM
N_CORES = 8
T = 2048          # tokens per core (= one batch element)
O = 2048          # d_out per core (half)
K = D_IN          # contraction
KH = K // 2       # K half resident in SBUF
NKT = KH // 128   # 16 k-tiles per half
TB = 1024         # token block
NTB = T // TB     # 2 token blocks
NTT_B = TB // 128  # 8 token tiles per block
OCW = 512         # o-chunk width (one PSUM bank)
NOC = O // OCW    # 4 o-chunks

# "f32r" (full-rate fp32, reduced-precision multiplies) or "f32" (exact fp32,
# 4 cycles/row). Flip via env for A/B testing; default chosen empirically.
COMPUTE = os.environ.get("LORA_KERNEL_COMPUTE", "f32r")

# Set by kernel() after a traced run (test.py reads it).
LAST_EXEC_TIME_NS = None
TRACE = False


class SplitDrainTileContext(tile.TileContext):
    """TileContext that splits multi-wait instructions for this walrus build.

    This walrus rejects instructions carrying >2 sync waits ("Too many sync
    wait commands"). Engine queues are in-order, so an instruction's waits
    can equivalently ride same-engine NOPs inserted just before it; we cap
    every instruction at one wait. Same treatment for the exit Drain.
    """

    _splitw_counter = 0

    def _split_excess_waits(self, ordered):
        for bb_name, insts in ordered.items():
            new_list = []
            changed = False
            for inst in insts:
                si = getattr(inst, "sync_info", None)
                eng = getattr(inst, "engine", mybir.EngineType.Unassigned)
                waits = list(si.on_wait) if si is not None and si.on_wait else []
                if len(waits) > 1 and eng != mybir.EngineType.Unassigned:
                    # keep register-valued waits (if any) on the original
                    movable = [w for w in waits if w.wait_reg is None]
                    pinned = [w for w in waits if w.wait_reg is not None]
                    keep = pinned + movable[-1:] if not pinned else pinned
                    move = movable[:-1] if not pinned else movable
                    for w in move:
                        SplitDrainTileContext._splitw_counter += 1
                        nop = bass_rust.InstNoOp(
                            name=f"tile_splitw_{SplitDrainTileContext._splitw_counter}",
                            ins=[],
                            outs=[],
                        )
                        nop.engine = eng
                        nop.sync_info = bass_rust.SyncInfo(
                            on_wait=[w], on_update=[]
                        )
                        new_list.append(nop)
                    inst.sync_info = bass_rust.SyncInfo(
                        on_wait=keep, on_update=list(si.on_update)
                    )
                    changed = True
                new_list.append(inst)
            if changed:
                insts[:] = new_list

    def _lower_ordered_insts(self, ordered):
        self._split_excess_waits(ordered)
        return super()._lower_ordered_insts(ordered)

    def _drain_and_barrier(self, tick_clock, wait_clock):
        g = tick_clock.global_clock
        for proc in range(len(g)):
            t = g[proc]
            if t <= 0:
                continue
            v = VectorClock()
            v.require_at_least(proc, t)
            nop = self.nc.sync.nop(nofuse=True)
            wait_clock.add_sem_waits(nop.ins, ScopedClock({None: v}))
        drain_inst = self.nc.sync.drain()
        wait_clock.add_sem_waits(
            drain_inst.ins, ScopedClock({None: g}), ScopedClock({None: g})
        )
        self.nc.all_engine_barrier()
        assert self.sems is not None
        popped = self.nc._tile_sem_poison_stack.pop()
        assert popped is self._sem_poison
        self.nc.clear_and_free_semaphores(list(self.sems.allocated().values()))
        self.nc.all_engine_barrier()


def _build_nc() -> bass.Bass:
    f32 = mybir.dt.float32
    mm_dt = mybir.dt.float32r if COMPUTE == "f32r" else f32

    nc = bass.Bass("TRN2", target_bir_lowering=False, debug=False)
    xT = nc.declare_dram_parameter("xT", [K, T], mm_dt, isOutput=False)
    wT = nc.declare_dram_parameter("wT", [K, O], mm_dt, isOutput=False)
    rT = nc.declare_dram_parameter("rT", [K, LORA_DIM], mm_dt, isOutput=False)
    lT = nc.declare_dram_parameter("lT", [LORA_DIM, O], f32, isOutput=False)
    biasr = nc.declare_dram_parameter("biasr", [128, O], f32, isOutput=False)
    out = nc.declare_dram_parameter("out", [T, O], f32, isOutput=True)

    with SplitDrainTileContext(nc) as tc:
        with (
            tc.tile_pool(name="xt", bufs=1) as xt_pool,
            tc.tile_pool(name="wt", bufs=2) as wt_pool,
            tc.tile_pool(name="consts", bufs=1) as const_pool,
            tc.tile_pool(name="outsb", bufs=2) as out_pool,
            tc.tile_pool(name="partsb", bufs=2) as part_pool,
            tc.tile_pool(name="psum", bufs=4, space="PSUM") as psum_pool,
            tc.tile_pool(name="psum1", bufs=2, space="PSUM") as psum1_pool,
            tc.tile_pool(name="dram", bufs=1, space="DRAM") as dram_pool,
        ):
            # constants: R^T (full K), [s*L^T; bias], xR^T-plus-ones row
            rt_sb = const_pool.tile([128, K // 128, LORA_DIM], mm_dt)
            nc.sync.dma_start(
                rt_sb[:], rT.rearrange("(ko p) j -> p ko j", p=128)
            )
            lt_sb = const_pool.tile([LORA_DIM, O], f32)
            nc.sync.dma_start(lt_sb[:], lT[:])
            bias_sb = const_pool.tile([128, O], f32)
            nc.sync.dma_start(bias_sb[:], biasr[:])
            xr = const_pool.tile([LORA_DIM, T], f32)

            partial = dram_pool.tile([T, O], f32)

            for kh in range(2):
                for tb in range(NTB):
                    xt = xt_pool.tile([128, NKT, TB], mm_dt, tag="xt")
                    x_src = xT[
                        kh * KH : (kh + 1) * KH, ts(tb, TB)
                    ].rearrange("(ko p) t -> p ko t", p=128)
                    for q in range(4):
                        nq = NKT // 4
                        nc.sync.dma_start(
                            xt[:, q * nq : (q + 1) * nq, :],
                            x_src[:, q * nq : (q + 1) * nq, :],
                        )

                    # stage A: xR^T[j, t] accumulated over kh into xr1 rows 0..15
                    for c in range(TB // 512):
                        tg = tb * (TB // 512) + c
                        p1 = psum1_pool.tile([LORA_DIM, 512], f32, tag="p1")
                        for k in range(NKT):
                            nc.tensor.matmul(
                                p1[:],
                                rt_sb[:, kh * NKT + k, :],
                                xt[:, k, ts(c, 512)],
                                start=(k == 0),
                                stop=(k == NKT - 1),
                            )
                        if kh == 0:
                            nc.vector.tensor_copy(xr[:, ts(tg, 512)], p1[:])
                        else:
                            nc.vector.tensor_add(
                                xr[:, ts(tg, 512)], xr[:, ts(tg, 512)], p1[:]
                            )

                    # main: psum[t-tile, o-chunk] over this K half
                    for oc in range(NOC):
                        wt = wt_pool.tile([128, NKT, OCW], mm_dt, tag="wt")
                        nc.sync.dma_start(
                            wt[:],
                            wT[kh * KH : (kh + 1) * KH, ts(oc, OCW)].rearrange(
                                "(ko p) o -> p ko o", p=128
                            ),
                        )
                        for tt in range(NTT_B):
                            gt = tb * NTT_B + tt  # global token tile
                            ps = psum_pool.tile([128, OCW], f32, tag="ps")
                            for k in range(NKT):
                                nc.tensor.matmul(
                                    ps[:],
                                    xt[:, k, ts(tt, 128)],
                                    wt[:, k, :],
                                    start=(k == 0),
                                    stop=(kh == 0 and k == NKT - 1),
                                )
                            if kh == 0:
                                # bias-add rides the psum->SBUF copy
                                ob = out_pool.tile([128, OCW], f32, tag="ob")
                                nc.vector.tensor_add(
                                    ob[:], ps[:], bias_sb[:, ts(oc, OCW)]
                                )
                                nc.sync.dma_start(
                                    partial[ts(gt, 128), ts(oc, OCW)], ob[:]
                                )
                            else:
                                # LoRA + bias ride the same accumulation group
                                # (kept exact fp32: tiny op count).
                                nc.tensor.matmul(
                                    ps[:],
                                    xr[:, ts(gt, 128)],
                                    lt_sb[:, ts(oc, OCW)],
                                    start=False,
                                    stop=True,
                                )
                                pb = part_pool.tile([128, OCW], f32, tag="pb")
                                nc.sync.dma_start(
                                    pb[:], partial[ts(gt, 128), ts(oc, OCW)]
                                )
                                ob = out_pool.tile([128, OCW], f32, tag="ob")
                                nc.vector.tensor_add(ob[:], ps[:], pb[:])
                                nc.sync.dma_start(
                                    out[ts(gt, 128), ts(oc, OCW)], ob[:]
                                )
    return nc


def kernel(**inputs: np.ndarray) -> np.ndarray:
    global LAST_EXEC_TIME_NS

    x = np.ascontiguousarray(np.asarray(inputs["x"], dtype=np.float32))
    weight = np.asarray(inputs["weight"], dtype=np.float32)
    bias = np.asarray(inputs["bias"], dtype=np.float32)
    lora_left = np.asarray(inputs["lora_left"], dtype=np.float32)
    lora_right = np.asarray(inputs["lora_right"], dtype=np.float32)

    # host-side shard + layout prep
    xT_shards = [np.ascontiguousarray(x[b].T) for b in range(B)]
    wT_halves = [
        np.ascontiguousarray(weight[oh * O : (oh + 1) * O, :].T) for oh in range(2)
    ]
    rT = np.ascontiguousarray(lora_right.T)
    lT_halves = [
        np.ascontiguousarray(LORA_SCALE * lora_left[oh * O : (oh + 1) * O, :].T)
        for oh in range(2)
    ]
    bias_halves = [
        np.ascontiguousarray(
            np.broadcast_to(bias[None, oh * O : (oh + 1) * O], (128, O))
        )
        for oh in range(2)
    ]

    in_maps = []
    for i in range(N_CORES):
        b, oh = i % B, i // B
        in_maps.append(
            {
                "xT": xT_shards[b],
                "wT": wT_halves[oh],
                "rT": rT,
                "lT": lT_halves[oh],
                "biasr": bias_halves[oh],
            }
        )

    nc = _build_nc()
    res = run_bass_kernel_spmd(
        nc, in_maps, core_ids=list(range(N_CORES)), trace=TRACE
    )
    LAST_EXEC_TIME_NS = res.exec_time_ns

    out = np.empty((B, S, D_OUT), dtype=np.float32)
    for i in range(N_CORES):
        b, oh = i % B, i // B
        out[b, :, oh * O : (oh + 1) * O] = res.results[i]["out"]
    return out
